# revision 6
# baseline (speedup 1.0000x reference)
import sys
sys.path.insert(0, '/opt/trn_rl_repo')
import numpy as np
import ml_dtypes

BF16 = ml_dtypes.bfloat16

N = 25000
E = 400000
NCORES = 8
NPC = 3200            # padded nodes per core (25 windows x 128)
NWIN = 25

_CACHE = {}


def _prep_weights(W_sc_s, W_sc_v, W1_s, W1_v, W_r1, W_r2, W2_s, W2_v):
    c_s, c_x = np.sin(np.pi / 8.0), np.cos(np.pi / 8.0)
    # lin1 (per-edge): x feature rows [s(64)|vx|vy|vz] -> g cols [s1(64)|v1x|v1y|v1z]
    Wnode = np.zeros((160, 160), np.float32)
    Wnode[0:64, 0:64] = W1_s / 8.0
    for c in range(3):
        Wnode[64 + 32 * c:96 + 32 * c, 64 + 32 * c:96 + 32 * c] = W1_v / np.sqrt(32.0)
    # self-connection: -> [y_s-pre(96) | y_v-pre c-major(96)] with c_s folded
    Wsc = np.zeros((160, 192), np.float32)
    Wsc[0:64, 0:96] = W_sc_s / 8.0 * c_s
    for c in range(3):
        Wsc[64 + 32 * c:96 + 32 * c, 96 + 32 * c:128 + 32 * c] = \
            W_sc_v / np.sqrt(32.0) * c_s
    Wr1p = (W_r1 / np.sqrt(12.0)).astype(np.float32)
    # radial-2: [100, 640], b-major blocks of 160 = [ss(64)|vvx(32)|vvy(32)|vvz(32)]
    w1 = W_r2[:, 0:64] / 10.0
    w2 = W_r2[:, 64:128] / 10.0
    w3 = W_r2[:, 128:160] / 10.0
    w4 = W_r2[:, 160:192] / 10.0
    w5 = W_r2[:, 192:224] / 10.0
    Wr2p = np.zeros((100, 640), np.float32)
    Wr2p[:, 0:64] = w1
    for c in range(3):
        Wr2p[:, 64 + 32 * c:96 + 32 * c] = w3
    for b in range(1, 4):
        o = 160 * b
        Wr2p[:, o:o + 64] = w2
        for cp in range(3):
            Wr2p[:, o + 64 + 32 * cp:o + 96 + 32 * cp] = w4 if cp == b - 1 else w5
    # lin2 over acc(640) -> yp cols [y_s(96) | y_v c-major(96) | pad(64)]
    k = c_x / 4.0
    ks = k / np.sqrt(96.0)
    kv = k / np.sqrt(128.0)
    eps = np.zeros((3, 3, 3), np.float32)
    eps[0, 1, 2] = eps[1, 2, 0] = eps[2, 0, 1] = 1.0
    eps[0, 2, 1] = eps[1, 0, 2] = eps[2, 1, 0] = -1.0
    W2p = np.zeros((640, 256), np.float32)
    W2p[0:64, 0:96] = W2_s[0:64] * ks                       # m0a
    for c in range(3):
        W2p[64 + 32 * c:96 + 32 * c, 96 + 32 * c:128 + 32 * c] = W2_v[64:96] * kv  # m1b
    for c in range(3):                                      # attr = ve_c
        o = 160 * (c + 1)
        W2p[o:o + 64, 96 + 32 * c:128 + 32 * c] = W2_v[0:64] * kv                  # m1a
        for cp in range(3):
            r = o + 64 + 32 * cp
            if cp == c:
                W2p[r:r + 32, 0:96] = W2_s[64:96] * ks / np.sqrt(3.0)              # m0b
            else:
                i = 3 - c - cp
                sgn = eps[i, cp, c]
                W2p[r:r + 32, 96 + 32 * i:128 + 32 * i] = \
                    W2_v[96:128] * kv * sgn / np.sqrt(2.0)                          # m1c
    return (Wnode.astype(BF16), Wsc.astype(BF16), Wr1p.astype(BF16),
            Wr2p.astype(BF16), W2p.astype(BF16))


def _prep_core(c, x, edge_src, edge_dst, edge_attr, edge_scalars, WT):
    xrow = np.concatenate([np.arange(64), 64 + 3 * np.arange(32),
                           65 + 3 * np.arange(32), 66 + 3 * np.arange(32)])
    own0 = c * NPC
    own_n = min(NPC, N - own0)

    sel = np.nonzero((edge_dst >= own0) & (edge_dst < own0 + own_n))[0]
    dl = edge_dst[sel] - own0
    win = dl >> 7
    order = np.argsort(win, kind='stable')
    sel = sel[order]
    dl = dl[order]
    win = win[order]

    TW = WT * 128
    EP = NWIN * TW
    xg_p = np.zeros((EP, 160), np.float32)
    es_p = np.zeros((EP, 12), np.float32)
    ea_p = np.zeros((EP, 4), np.float32)
    col_p = np.full(EP, -1.0, np.float32)
    for w in range(NWIN):
        m = win == w
        ew = sel[m]
        k = ew.size
        o = w * TW
        xg_p[o:o + k] = x[edge_src[ew]][:, xrow]
        es_p[o:o + k] = edge_scalars[ew]
        ea_p[o:o + k] = edge_attr[ew]
        col_p[o:o + k] = (dl[m] & 127).astype(np.float32)

    T = EP // 128
    xgT = np.ascontiguousarray(xg_p.T).astype(BF16)
    esT = np.ascontiguousarray(es_p.T).astype(BF16)
    eaT = np.ascontiguousarray(
        ea_p.reshape(T, 128, 4).transpose(1, 0, 2).reshape(128, T * 4)).astype(BF16)
    dstT = np.ascontiguousarray(col_p.reshape(T, 128).T)
    xsc = np.zeros((NPC, 160), np.float32)
    xsc[:own_n] = x[own0:own0 + own_n][:, xrow]
    xscT = np.ascontiguousarray(xsc.T).astype(BF16)
    return dict(xgT=xgT, esT=esT, eaT=eaT, dstT=dstT, xscT=xscT)


def _build_program(WT):
    import concourse.bass as bass
    import concourse.tile as tile
    from concourse import bacc, mybir

    f32 = mybir.dt.float32
    bf16 = mybir.dt.bfloat16
    i32 = mybir.dt.int32
    AF = mybir.ActivationFunctionType
    MUL = mybir.AluOpType.mult
    TW = WT * 128
    EP = NWIN * TW

    nc = bacc.Bacc("TRN2", num_devices=NCORES, debug=False)
    xgT_ap = nc.dram_tensor("xgT", [160, EP], bf16, kind="ExternalInput").ap()
    esT_ap = nc.dram_tensor("esT", [12, EP], bf16, kind="ExternalInput").ap()
    eaT_ap = nc.dram_tensor("eaT", [128, (EP // 128) * 4], bf16,
                            kind="ExternalInput").ap()
    dstT_ap = nc.dram_tensor("dstT", [128, EP // 128], f32, kind="ExternalInput").ap()
    xscT_ap = nc.dram_tensor("xscT", [160, NPC], bf16, kind="ExternalInput").ap()
    Wnode_ap = nc.dram_tensor("Wnode", [160, 160], bf16, kind="ExternalInput").ap()
    Wsc_ap = nc.dram_tensor("Wsc", [160, 192], bf16, kind="ExternalInput").ap()
    Wr1_ap = nc.dram_tensor("Wr1p", [12, 100], bf16, kind="ExternalInput").ap()
    Wr2_ap = nc.dram_tensor("Wr2p", [100, 640], bf16, kind="ExternalInput").ap()
    W2p_ap = nc.dram_tensor("W2p", [640, 256], bf16, kind="ExternalInput").ap()
    out_ap = nc.dram_tensor("out", [NPC, 160], f32, kind="ExternalOutput").ap()

    with tile.TileContext(nc) as tc:
        from contextlib import ExitStack
        with ExitStack() as ctx:
            wpool = ctx.enter_context(tc.tile_pool(name="weights", bufs=1))

            wn1 = wpool.tile([128, 160], bf16)
            wn2 = wpool.tile([32, 160], bf16)
            ws1 = wpool.tile([128, 192], bf16)
            ws2 = wpool.tile([32, 192], bf16)
            wr1 = wpool.tile([12, 100], bf16)
            wr2 = wpool.tile([100, 640], bf16)
            w2p = [wpool.tile([128, 256], bf16, tag=f"w2p{j}", name=f"w2p{j}")
                   for j in range(5)]
            nc.sync.dma_start(wn1[:], Wnode_ap[0:128, :])
            nc.sync.dma_start(wn2[:], Wnode_ap[128:160, :])
            nc.sync.dma_start(ws1[:], Wsc_ap[0:128, :])
            nc.sync.dma_start(ws2[:], Wsc_ap[128:160, :])
            nc.sync.dma_start(wr1[:], Wr1_ap[:])
            nc.sync.dma_start(wr2[:], Wr2_ap[:])
            for j in range(5):
                nc.sync.dma_start(w2p[j][:], W2p_ap[j * 128:(j + 1) * 128, :])

            ioti = wpool.tile([128, 128], i32)
            iotf = wpool.tile([128, 128], f32)
            iotci = wpool.tile([128, 1], i32)
            iotcf = wpool.tile([128, 1], f32)
            ident = wpool.tile([128, 128], bf16)
            nc.gpsimd.iota(ioti[:], pattern=[[1, 128]], base=0, channel_multiplier=0)
            nc.vector.tensor_copy(iotf[:], ioti[:])
            nc.gpsimd.iota(iotci[:], pattern=[[0, 1]], base=0, channel_multiplier=1)
            nc.vector.tensor_copy(iotcf[:], iotci[:])
            nc.vector.tensor_scalar(ident[:], iotf[:], iotcf[:], None,
                                    op0=mybir.AluOpType.is_equal)
            scN = wpool.tile([128, NWIN * 192], bf16)

            # Phase SC: self-connection for own nodes
            with tc.tile_pool(name="xsa", bufs=2) as xsa, \
                 tc.tile_pool(name="xsb", bufs=2) as xsb, \
                 tc.tile_pool(name="scp", bufs=2, space="PSUM") as scp:
                for b in range(NWIN):
                    xs1 = xsa.tile([128, 128], bf16)
                    xs2 = xsb.tile([32, 128], bf16)
                    nc.sync.dma_start(xs1[:], xscT_ap[0:128, b * 128:(b + 1) * 128])
                    nc.sync.dma_start(xs2[:], xscT_ap[128:160, b * 128:(b + 1) * 128])
                    st = scp.tile([128, 192], f32)
                    nc.tensor.matmul(st[:], xs1[:], ws1[:], start=True, stop=False)
                    nc.tensor.matmul(st[:], xs2[:], ws2[:], start=False, stop=True)
                    nc.scalar.activation(scN[:, b * 192:(b + 1) * 192], st[:], AF.Copy)

            # Phase B: edges
            xg1P = ctx.enter_context(tc.tile_pool(name="xg1", bufs=2))
            xg2P = ctx.enter_context(tc.tile_pool(name="xg2", bufs=2))
            esP = ctx.enter_context(tc.tile_pool(name="esw", bufs=2))
            eaP = ctx.enter_context(tc.tile_pool(name="eaw", bufs=2))
            dsP = ctx.enter_context(tc.tile_pool(name="dsw", bufs=2))
            hsP = ctx.enter_context(tc.tile_pool(name="hs", bufs=2))
            gP = ctx.enter_context(tc.tile_pool(name="gp", bufs=1, space="PSUM"))
            gsP = ctx.enter_context(tc.tile_pool(name="gs", bufs=2))
            hpP = ctx.enter_context(tc.tile_pool(name="hp", bufs=1, space="PSUM"))
            wpP = ctx.enter_context(tc.tile_pool(name="wp", bufs=1, space="PSUM"))
            wsP = ctx.enter_context(tc.tile_pool(name="wsb", bufs=2))
            mP = ctx.enter_context(tc.tile_pool(name="mid", bufs=2))
            ohP = ctx.enter_context(tc.tile_pool(name="oh", bufs=2))
            accP = ctx.enter_context(tc.tile_pool(name="acc", bufs=1, space="PSUM"))
            tlP = ctx.enter_context(tc.tile_pool(name="tail", bufs=2))
            tpsP = ctx.enter_context(tc.tile_pool(name="tps", bufs=1, space="PSUM"))
            ypP = ctx.enter_context(tc.tile_pool(name="yp", bufs=1, space="PSUM"))
            oP = ctx.enter_context(tc.tile_pool(name="outs", bufs=2))

            for w in range(NWIN):
                xg1w = xg1P.tile([128, TW], bf16)
                xg2w = xg2P.tile([32, TW], bf16)
                nc.sync.dma_start(xg1w[:], xgT_ap[0:128, w * TW:(w + 1) * TW])
                nc.sync.dma_start(xg2w[:], xgT_ap[128:160, w * TW:(w + 1) * TW])
                esw = esP.tile([12, TW], bf16)
                nc.sync.dma_start(esw[:], esT_ap[:, w * TW:(w + 1) * TW])
                eaw = eaP.tile([128, 4 * WT], bf16)
                nc.sync.dma_start(eaw[:], eaT_ap[:, w * 4 * WT:(w + 1) * 4 * WT])
                dsw = dsP.tile([128, WT], f32)
                nc.sync.dma_start(dsw[:], dstT_ap[:, w * WT:(w + 1) * WT])

                hsb = hsP.tile([100, TW], bf16)
                for j in range(TW // 384):
                    hp = hpP.tile([100, 384], f32)
                    nc.tensor.matmul(hp[:], wr1[:], esw[:, j * 384:(j + 1) * 384],
                                     start=True, stop=True)
                    nc.scalar.activation(hsb[:, j * 384:(j + 1) * 384], hp[:], AF.Silu)

                acc0 = accP.tile([128, 320], f32, tag="acc0")
                acc1 = accP.tile([128, 320], f32, tag="acc1")
                for t in range(WT):
                    gp = gP.tile([128, 160], f32)
                    nc.tensor.matmul(gp[:], xg1w[:, t * 128:(t + 1) * 128], wn1[:],
                                     start=True, stop=False)
                    nc.tensor.matmul(gp[:], xg2w[:, t * 128:(t + 1) * 128], wn2[:],
                                     start=False, stop=True)
                    gsb = gsP.tile([128, 160], bf16)
                    nc.vector.tensor_copy(gsb[:], gp[:])
                    wp0 = wpP.tile([128, 320], f32, tag="wp0")
                    wp1 = wpP.tile([128, 320], f32, tag="wp1")
                    nc.tensor.matmul(wp0[:], hsb[:, t * 128:(t + 1) * 128],
                                     wr2[:, 0:320], start=True, stop=True)
                    nc.tensor.matmul(wp1[:], hsb[:, t * 128:(t + 1) * 128],
                                     wr2[:, 320:640], start=True, stop=True)
                    wsb = wsP.tile([128, 640], bf16)
                    nc.scalar.activation(wsb[:, 0:320], wp0[:], AF.Copy)
                    nc.scalar.activation(wsb[:, 320:640], wp1[:], AF.Copy)
                    mid = mP.tile([128, 640], bf16)
                    for b in range(4):
                        nc.vector.scalar_tensor_tensor(
                            mid[:, b * 160:(b + 1) * 160],
                            wsb[:, b * 160:(b + 1) * 160],
                            eaw[:, 4 * t + b:4 * t + b + 1],
                            gsb[:], op0=MUL, op1=MUL)
                    oh = ohP.tile([128, 128], bf16)
                    nc.vector.tensor_scalar(oh[:], iotf[:], dsw[:, t:t + 1], None,
                                            op0=mybir.AluOpType.is_equal)
                    nc.tensor.matmul(acc0[:], oh[:], mid[:, 0:320],
                                     start=(t == 0), stop=(t == WT - 1))
                    nc.tensor.matmul(acc1[:], oh[:], mid[:, 320:640],
                                     start=(t == 0), stop=(t == WT - 1))

                # window tail: lin2 + sc + gate
                asb = tlP.tile([128, 640], bf16, tag="asb")
                nc.scalar.activation(asb[:, 0:320], acc0[:], AF.Copy)
                nc.scalar.activation(asb[:, 320:640], acc1[:], AF.Copy)
                yp = ypP.tile([128, 256], f32)
                for j in range(5):
                    tp = tpsP.tile([128, 128], bf16, tag="tp")
                    nc.tensor.transpose(tp[:], asb[:, j * 128:(j + 1) * 128], ident[:])
                    ts = tlP.tile([128, 128], bf16, tag="ts")
                    nc.scalar.activation(ts[:], tp[:], AF.Copy)
                    nc.tensor.matmul(yp[:], ts[:], w2p[j][:],
                                     start=(j == 0), stop=(j == 4))
                ysb = tlP.tile([128, 192], bf16, tag="ysb")
                nc.scalar.activation(ysb[:], yp[:, 0:192], AF.Copy)
                y2 = tlP.tile([128, 192], bf16, tag="y2")
                nc.vector.tensor_add(y2[:], ysb[:], scN[:, w * 192:(w + 1) * 192])
                outt = oP.tile([128, 160], f32, tag="outt")
                gtl = oP.tile([128, 32], bf16, tag="gtl")
                sgo = oP.tile([128, 64], bf16, tag="sgo")
                nc.scalar.activation(sgo[:], y2[:, 0:64], AF.Sigmoid)
                nc.vector.tensor_mul(outt[:, 0:64], y2[:, 0:64], sgo[:])
                nc.scalar.activation(gtl[:], y2[:, 64:96], AF.Sigmoid)
                for c in range(3):
                    nc.vector.tensor_mul(outt[:, 64 + 32 * c:96 + 32 * c],
                                         y2[:, 96 + 32 * c:128 + 32 * c], gtl[:])
                nc.sync.dma_start(out_ap[w * 128:(w + 1) * 128, :], outt[:])

    nc.compile()
    return nc


def kernel(x, z, edge_src, edge_dst, edge_attr, edge_scalars,
           W_sc_s, W_sc_v, W1_s, W1_v, W_r1, W_r2, W2_s, W2_v):
    from concourse import bass_utils
    x = np.asarray(x, np.float32)
    edge_src = np.asarray(edge_src, np.int64)
    edge_dst = np.asarray(edge_dst, np.int64)
    edge_attr = np.asarray(edge_attr, np.float32)
    edge_scalars = np.asarray(edge_scalars, np.float32)

    # uniform tiles-per-window across all cores/windows (SPMD: one program)
    counts = np.zeros((NCORES, NWIN), np.int64)
    cw = (edge_dst // NPC) * NWIN + (edge_dst % NPC) // 128
    u, ct = np.unique(cw, return_counts=True)
    counts.flat[u] = ct
    WT = int(np.ceil(counts.max() / 128.0))
    WT = ((WT + 2) // 3) * 3  # multiple of 3 for 384-wide radial matmuls

    key = WT
    if key not in _CACHE:
        _CACHE[key] = _build_program(WT)
    nc = _CACHE[key]

    Wnode, Wsc, Wr1p, Wr2p, W2p = _prep_weights(
        np.asarray(W_sc_s, np.float32), np.asarray(W_sc_v, np.float32),
        np.asarray(W1_s, np.float32), np.asarray(W1_v, np.float32),
        np.asarray(W_r1, np.float32), np.asarray(W_r2, np.float32),
        np.asarray(W2_s, np.float32), np.asarray(W2_v, np.float32))

    in_maps = []
    for c in range(NCORES):
        m = _prep_core(c, x, edge_src, edge_dst, edge_attr, edge_scalars, WT)
        m.update(Wnode=Wnode, Wsc=Wsc, Wr1p=Wr1p, Wr2p=Wr2p, W2p=W2p)
        in_maps.append(m)

    res = bass_utils.run_bass_kernel_spmd(nc, in_maps, core_ids=list(range(NCORES)))
    parts = []
    for c in range(NCORES):
        own_n = min(NPC, N - c * NPC)
        parts.append(res.results[c]["out"][:own_n])
    full = np.concatenate(parts, axis=0)
    out = np.empty((N, 160), np.float32)
    out[:, 0:64] = full[:, 0:64]
    # device gated layout is c-major [32c+u]; reference wants u-major [3u+c]
    out[:, 64:160] = full[:, 64:160].reshape(N, 3, 32).transpose(0, 2, 1).reshape(N, 96)
    return out
